# revision 1
# baseline (speedup 1.0000x reference)
"""Trainium2 Bass kernel for CombinedAdvancedLoss (focal + contrastive +
circularity + consensus), data-parallel over 8 NeuronCores.

Sharding: batch dim B=32 -> 4 items per core for logits/target/masks/
method_preds. features (1024x512) are passed to each core ROLLED by
-core*128 rows, so every core computes the same SPMD program on "its" 128
rows of the 1024x1024 similarity matrix (row sums / logsumexp are invariant
to the column permutation the roll induces; the diagonal lands in local
column block 0 and the positive pair in block 4).

Each core emits a [1,32] vector of linear partial sums; the host combines
them (the only nonlinear cross-core math - IoU ratios and the circularity
formula - acts on a handful of scalars).
"""

import sys

for _p in ("/opt/trn_rl_repo",):
    if _p not in sys.path:
        sys.path.insert(0, _p)

import numpy as np
import ml_dtypes

import concourse.bass as bass
import concourse.tile as tile
from concourse import mybir
from concourse.bass_utils import run_bass_kernel_spmd

import bass_rust as _bass_rust

# ---------------------------------------------------------------------------
# The walrus build in this container rejects >2 sync waits per instruction.
# Post-pass: hoist excess waits onto inserted same-engine NoOps.
_WAIT_CAP = 1


def _split_sync_waits(nc):
    n = 0
    for fn in nc.m.functions:
        for blk in fn.blocks:
            insts = blk.instructions
            i = 0
            while i < len(insts):
                inst = insts[i]
                si = inst.sync_info
                if si is not None and len(si.on_wait) > _WAIT_CAP:
                    waits = list(si.on_wait)
                    keep = waits[-_WAIT_CAP:]
                    extra = waits[:-_WAIT_CAP]
                    nops = []
                    for j in range(0, len(extra), _WAIT_CAP):
                        nop = mybir.InstDrain(
                            name=f"I-wsplit-{n}", engine=inst.engine)
                        n += 1
                        nop.sync_info = _bass_rust.SyncInfo(
                            on_wait=extra[j:j + _WAIT_CAP], on_update=[])
                        nops.append(nop)
                    inst.sync_info = _bass_rust.SyncInfo(
                        on_wait=keep, on_update=list(si.on_update))
                    for k, nop in enumerate(nops):
                        insts.insert(i + k, nop)
                    i += len(nops)
                i += 1
# ---------------------------------------------------------------------------

F32 = mybir.dt.float32
BF16 = mybir.dt.bfloat16
I32 = mybir.dt.int32
AF = mybir.ActivationFunctionType
OP = mybir.AluOpType
AX = mybir.AxisListType

NCORES = 8
B, C, H, W = 32, 8, 256, 256
BP = B // NCORES          # batch items per core (4)
HW = H * W                # 65536
FD = BP * HW // 128       # free dim of a full-core tile (2048)
XB = HW // 128            # free dim of one plane slice (512)
BF, DF = 1024, 512        # features shape
TEMP = 0.07
GAMMA_SCALE = 0.25        # ALPHA (0.25 for every class) * W_FOCAL
NPART = 32                # width of the per-core partials vector

# partials vector layout
K_FOCAL = 0               # sum 0.25*(1-p)^2 * ce
K_CONTRAST = 1            # sum (lse - pos) over this core's 128 rows
K_AREA = 2                # 4 cols: per-b mask area
K_EX = 6                  # 4 cols: per-b sum |dm/dh|
K_EY = 10                 # 8 cols: per-b (x2 chunks) sum |dm/dw|
K_S = 18                  # 3 cols: per-method sum of preds
K_I = 21                  # 3 cols: per-pair sum pi*pj  (01, 02, 12)
K_EXB = 26                # 4 cols: per-b boundary |m[128]-m[127]|


def _build_nc():
    nc = bass.Bass()

    lg = nc.declare_dram_parameter("lg", [BP, C, 128, XB], F32, isOutput=False)
    tg = nc.declare_dram_parameter("tg", [BP, 128, XB], I32, isOutput=False)
    mk = nc.declare_dram_parameter("mk", [BP, 2, 128, 256], F32, isOutput=False)
    mp = nc.declare_dram_parameter("mp", [3, BP, 128, XB], F32, isOutput=False)
    ft = nc.declare_dram_parameter("ft", [8, 128, DF], F32, isOutput=False)
    idf = nc.declare_dram_parameter("idf", [128, 128], F32, isOutput=False)
    idb = nc.declare_dram_parameter("idb", [128, 128], BF16, isOutput=False)
    zm = nc.declare_dram_parameter("zm", [128, 128], F32, isOutput=False)
    mb = nc.declare_dram_parameter("mb", [1, BP, 2, 256], F32, isOutput=False)
    out = nc.declare_dram_parameter("partials", [1, NPART], F32, isOutput=True)

    with tile.TileContext(nc) as tc:
        _emit(nc, tc, lg, tg, mk, mp, ft, idf, idb, zm, mb, out)
    _split_sync_waits(nc)
    return nc


def _emit(nc, tc, lg, tg, mk, mp, ft, idf, idb, zm, mb, out):
    from contextlib import ExitStack

    ctx = ExitStack()
    with ctx:
        singles = ctx.enter_context(tc.tile_pool(name="singles", bufs=1))
        lpool = ctx.enter_context(tc.tile_pool(name="lpool", bufs=3))
        qpool = ctx.enter_context(tc.tile_pool(name="qpool", bufs=3))
        mqpool = ctx.enter_context(tc.tile_pool(name="mqpool", bufs=3))
        spool = ctx.enter_context(tc.tile_pool(name="spool", bufs=2))
        ppool = ctx.enter_context(tc.tile_pool(name="ppool", bufs=1))
        fpool = ctx.enter_context(tc.tile_pool(name="fpool", bufs=1))
        scratch = ctx.enter_context(tc.tile_pool(name="scratch", bufs=1))
        tiny = ctx.enter_context(tc.tile_pool(name="tiny", bufs=1))
        cpool = ctx.enter_context(tc.tile_pool(name="cpool", bufs=2))
        pst = ctx.enter_context(tc.tile_pool(name="pst", bufs=2, space="PSUM"))
        pss = ctx.enter_context(tc.tile_pool(name="pss", bufs=1, space="PSUM"))
        psc = ctx.enter_context(tc.tile_pool(name="psc", bufs=2, space="PSUM"))
        psf = ctx.enter_context(tc.tile_pool(name="psf", bufs=1, space="PSUM"))

        # constants + accumulator
        ones = singles.tile([128, 1], F32)
        nc.vector.memset(ones, 1.0)
        acc = singles.tile([128, NPART], F32)
        nc.vector.memset(acc, 0.0)
        ident_f = singles.tile([128, 128], F32)
        nc.sync.dma_start(out=ident_f, in_=idf[:, :])
        ident_b = singles.tile([128, 128], BF16)
        nc.sync.dma_start(out=ident_b, in_=idb[:, :])
        zm_t = singles.tile([128, 128], F32)
        nc.sync.dma_start(out=zm_t, in_=zm[:, :])

        # ----------------- focal loss partials -----------------
        tg_t = singles.tile([128, BP, XB], I32)
        nc.sync.dma_start(out=tg_t, in_=tg.rearrange("b p x -> p b x"))
        tg_b = singles.tile([128, FD], BF16)
        nc.vector.tensor_copy(out=tg_b, in_=tg_t.rearrange("p b x -> p (b x)"))

        s_acc = None
        pt_acc = None
        q_prev = None
        mq_prev = None
        for c in range(C):
            l_c = lpool.tile([128, BP, XB], F32, tag="l")
            nc.sync.dma_start(out=l_c, in_=lg[:, c].rearrange("b p x -> p b x"))
            q_c = qpool.tile([128, FD], BF16, tag="q")
            nc.scalar.activation(
                out=q_c, in_=l_c.rearrange("p b x -> p (b x)"), func=AF.Exp
            )
            mq_c = mqpool.tile([128, FD], BF16, tag="mq")
            nc.vector.scalar_tensor_tensor(
                out=mq_c, in0=tg_b, scalar=float(c), in1=q_c,
                op0=OP.is_equal, op1=OP.mult,
            )
            if c == 0:
                q_prev, mq_prev = q_c, mq_c
            elif c == 1:
                s_acc = spool.tile([128, FD], BF16, tag="s")
                nc.vector.tensor_tensor(out=s_acc, in0=q_prev, in1=q_c, op=OP.add)
                pt_acc = spool.tile([128, FD], BF16, tag="pt")
                nc.vector.tensor_tensor(out=pt_acc, in0=mq_prev, in1=mq_c, op=OP.add)
                q_prev = mq_prev = None
            else:
                s_new = spool.tile([128, FD], BF16, tag="s")
                nc.vector.tensor_tensor(out=s_new, in0=s_acc, in1=q_c, op=OP.add)
                s_acc = s_new
                pt_new = spool.tile([128, FD], BF16, tag="pt")
                nc.vector.tensor_tensor(out=pt_new, in0=pt_acc, in1=mq_c, op=OP.add)
                pt_acc = pt_new

        ln_s = scratch.tile([128, FD], BF16, tag="lns")
        nc.scalar.activation(out=ln_s, in_=s_acc, func=AF.Ln)
        ln_pt = scratch.tile([128, FD], BF16, tag="lnpt")
        nc.scalar.activation(out=ln_pt, in_=pt_acc, func=AF.Ln)
        ce = scratch.tile([128, FD], BF16, tag="ce")
        nc.vector.tensor_tensor(out=ce, in0=ln_s, in1=ln_pt, op=OP.subtract)
        p_t = scratch.tile([128, FD], BF16, tag="p")
        nc.scalar.activation(out=p_t, in_=ce, func=AF.Exp, scale=-1.0)
        u_t = scratch.tile([128, FD], BF16, tag="u")
        nc.vector.tensor_scalar(
            out=u_t, in0=p_t, scalar1=-1.0, scalar2=1.0, op0=OP.mult, op1=OP.add
        )
        v_t = scratch.tile([128, FD], BF16, tag="v")
        nc.vector.tensor_tensor(out=v_t, in0=u_t, in1=u_t, op=OP.mult)
        w_t = scratch.tile([128, FD], BF16, tag="wt")
        nc.vector.tensor_tensor(out=w_t, in0=v_t, in1=ce, op=OP.mult)
        w_junk = scratch.tile([128, FD], BF16, tag="wj")
        nc.vector.tensor_scalar(
            out=w_junk, in0=w_t, scalar1=GAMMA_SCALE, scalar2=0.0,
            op0=OP.mult, op1=OP.add, accum_out=acc[:, K_FOCAL:K_FOCAL + 1],
        )

        # ----------------- consensus partials -----------------
        p_tiles = []
        for i in range(3):
            p_i = ppool.tile([128, BP, XB], F32, tag=f"mp{i}")
            nc.sync.dma_start(out=p_i, in_=mp[i].rearrange("b p x -> p b x"))
            p_tiles.append(p_i)
            sj = scratch.tile([128, FD], BF16, tag="wj")
            nc.vector.tensor_scalar(
                out=sj, in0=p_i.rearrange("p b x -> p (b x)"), scalar1=1.0,
                scalar2=0.0, op0=OP.mult, op1=OP.add,
                accum_out=acc[:, K_S + i:K_S + i + 1],
            )
        for k, (i, j) in enumerate(((0, 1), (0, 2), (1, 2))):
            ij = scratch.tile([128, FD], BF16, tag="wt")
            nc.vector.tensor_tensor(
                out=ij, in0=p_tiles[i].rearrange("p b x -> p (b x)"),
                in1=p_tiles[j].rearrange("p b x -> p (b x)"), op=OP.mult,
            )
            ij2 = scratch.tile([128, FD], BF16, tag="wj")
            nc.vector.tensor_scalar(
                out=ij2, in0=ij, scalar1=1.0, scalar2=0.0,
                op0=OP.mult, op1=OP.add,
                accum_out=acc[:, K_I + k:K_I + k + 1],
            )

        # ----------------- circularity partials -----------------
        m_t = singles.tile([128, BP, 2, 256], F32)
        nc.sync.dma_start(out=m_t, in_=mk.rearrange("b c p w -> p b c w"))
        for b in range(BP):
            ps_b = psc.tile([128, 2, 256], F32, tag="circ")
            nc.tensor.matmul(
                out=ps_b, lhsT=zm_t, rhs=m_t[:, b], start=True, stop=True
            )
            nc.vector.tensor_reduce(
                out=acc[:, K_EX + b:K_EX + b + 1], in_=ps_b,
                axis=AX.XY, op=OP.add, apply_absolute_value=True,
            )
            aj = scratch.tile([128, XB], BF16, tag="actj")
            nc.scalar.activation(
                out=aj, in_=m_t[:, b].rearrange("p c w -> p (c w)"), func=AF.Copy,
                accum_out=acc[:, K_AREA + b:K_AREA + b + 1],
            )
        mb_t = singles.tile([1, BP, 2, 256], F32)
        nc.sync.dma_start(out=mb_t, in_=mb[:, :, :, :])
        d_bnd = cpool.tile([1, BP, 256], BF16, tag="dbnd")
        nc.vector.tensor_tensor(
            out=d_bnd, in0=mb_t[:, :, 1], in1=mb_t[:, :, 0], op=OP.subtract
        )
        nc.vector.tensor_reduce(
            out=acc[0:1, K_EXB:K_EXB + BP], in_=d_bnd,
            axis=AX.X, op=OP.add, apply_absolute_value=True,
        )
        d_y = singles.tile([128, BP, 2, 255], BF16)
        nc.vector.tensor_tensor(
            out=d_y, in0=m_t[:, :, :, 1:256], in1=m_t[:, :, :, 0:255],
            op=OP.subtract,
        )
        nc.vector.tensor_reduce(
            out=acc[:, K_EY:K_EY + 8].rearrange("p (b c) -> p b c", b=BP),
            in_=d_y, axis=AX.X, op=OP.add, apply_absolute_value=True,
        )

        # ----------------- contrastive partials -----------------
        f_t = fpool.tile([128, 8, DF], F32)
        nc.sync.dma_start(out=f_t, in_=ft.rearrange("k p d -> p k d"))
        ss = tiny.tile([128, 8], F32, tag="ss")
        for k in range(8):
            fsq = scratch.tile([128, DF], BF16, tag="actj")
            nc.scalar.activation(
                out=fsq, in_=f_t[:, k], func=AF.Square,
                accum_out=ss[:, k:k + 1],
            )
        # rsqrt via exp(-0.5*ln(ss)) (stays in the exp/ln table set),
        # then one Newton step y' = y*(1.5 - 0.5*ss*y^2)
        lns_t = tiny.tile([128, 8], F32, tag="lnss")
        nc.scalar.activation(out=lns_t, in_=ss, func=AF.Ln)
        inv0 = tiny.tile([128, 8], F32, tag="inv0")
        nc.scalar.activation(out=inv0, in_=lns_t, func=AF.Exp, scale=-0.5)
        t1 = tiny.tile([128, 8], F32, tag="t1")
        nc.vector.tensor_tensor(out=t1, in0=inv0, in1=inv0, op=OP.mult)
        t2 = tiny.tile([128, 8], F32, tag="t2")
        nc.vector.tensor_tensor(out=t2, in0=t1, in1=ss, op=OP.mult)
        t3 = tiny.tile([128, 8], F32, tag="t3")
        nc.vector.tensor_scalar(
            out=t3, in0=t2, scalar1=-0.5, scalar2=1.5, op0=OP.mult, op1=OP.add
        )
        inv = tiny.tile([128, 8], F32, tag="inv")
        nc.vector.tensor_tensor(out=inv, in0=inv0, in1=t3, op=OP.mult)

        fn = fpool.tile([128, 8, DF], BF16)
        for k in range(8):
            nc.vector.tensor_scalar(
                out=fn[:, k], in0=f_t[:, k], scalar1=inv[:, k:k + 1],
                scalar2=None, op0=OP.mult,
            )
        ftr = [
            fpool.tile([128, 8, 128], BF16, tag=f"ftr{dc}", name=f"ftr{dc}")
            for dc in range(4)
        ]
        for k in range(8):
            for dc in range(4):
                ps_t = pst.tile([128, 128], BF16, tag="tr")
                nc.tensor.transpose(
                    out=ps_t, in_=fn[:, k, dc * 128:(dc + 1) * 128],
                    identity=ident_b,
                )
                nc.vector.tensor_copy(out=ftr[dc][:, k], in_=ps_t)
        sim = []
        for half in range(2):
            ps_h = pss.tile([128, 512], F32, tag=f"sim{half}")
            for dc in range(4):
                nc.tensor.matmul(
                    out=ps_h,
                    lhsT=ftr[dc][:, 0],
                    rhs=ftr[dc].rearrange("p k x -> p (k x)")[
                        :, half * 512:(half + 1) * 512],
                    start=(dc == 0), stop=(dc == 3),
                )
            sim.append(ps_h)
        # rolled features: diagonal = local column block 0, positive = block 4
        nc.vector.scalar_tensor_tensor(
            out=sim[0][:, 0:128], in0=ident_f, scalar=-1e4,
            in1=sim[0][:, 0:128], op0=OP.mult, op1=OP.add,
        )
        rsum = tiny.tile([128, 2], F32, tag="rsum")
        for half in range(2):
            e_h = scratch.tile([128, 512], BF16, tag="actj")
            nc.scalar.activation(
                out=e_h, in_=sim[half], func=AF.Exp, scale=1.0 / TEMP,
                accum_out=rsum[:, half:half + 1],
            )
        rtot = tiny.tile([128, 1], F32, tag="rtot")
        nc.vector.tensor_tensor(
            out=rtot, in0=rsum[:, 0:1], in1=rsum[:, 1:2], op=OP.add
        )
        lse = tiny.tile([128, 1], F32, tag="lse")
        nc.scalar.activation(out=lse, in_=rtot, func=AF.Ln)
        posj = scratch.tile([128, 128], F32, tag="posj")
        pos = tiny.tile([128, 1], F32, tag="pos")
        nc.vector.tensor_tensor(
            out=posj, in0=sim[1][:, 0:128], in1=ident_f, op=OP.mult
        )
        posj2 = scratch.tile([128, 128], BF16, tag="posj2")
        nc.vector.tensor_scalar(
            out=posj2, in0=posj, scalar1=1.0 / TEMP, scalar2=0.0,
            op0=OP.mult, op1=OP.add, accum_out=pos,
        )
        nc.vector.tensor_tensor(
            out=acc[:, K_CONTRAST:K_CONTRAST + 1], in0=lse, in1=pos,
            op=OP.subtract,
        )

        # ----------------- partition-reduce + store -----------------
        pfin = psf.tile([1, NPART], F32)
        nc.tensor.matmul(out=pfin, lhsT=ones, rhs=acc, start=True, stop=True)
        out_t = tiny.tile([1, NPART], F32, tag="outt")
        nc.vector.tensor_copy(out=out_t, in_=pfin)
        nc.sync.dma_start(out=out[:, :], in_=out_t)


def _zmat():
    ident = np.eye(128, dtype=np.float32)
    z = np.roll(ident, -1, axis=0) - ident
    z[:, 127] = 0.0
    return np.ascontiguousarray(z)


def _host_inputs(logits, target, features, masks, method_preds):
    """Slice/reshape full inputs into per-core input maps."""
    ident = np.eye(128, dtype=np.float32)
    consts = {
        "idf": ident,
        "idb": ident.astype(ml_dtypes.bfloat16),
        # zm = P127 @ (Cyc - I): row-diff matrix with output row 127 zeroed
        "zm": _zmat(),
    }
    in_maps = []
    for c in range(NCORES):
        b0 = c * BP
        in_maps.append({
            "lg": np.ascontiguousarray(
                logits[b0:b0 + BP].reshape(BP, C, 128, XB)),
            "tg": np.ascontiguousarray(
                target[b0:b0 + BP].reshape(BP, 128, XB)),
            "mk": np.ascontiguousarray(
                masks[b0:b0 + BP, 0].reshape(BP, 2, 128, 256)),
            "mp": np.ascontiguousarray(
                method_preds[:, b0:b0 + BP].reshape(3, BP, 128, XB)),
            "mb": np.ascontiguousarray(
                masks[b0:b0 + BP, 0, 127:129, :].reshape(1, BP, 2, 256)),
            "ft": np.ascontiguousarray(
                np.roll(features, -c * 128, axis=0).reshape(8, 128, DF)),
            **consts,
        })
    return in_maps


def _combine(partials):
    """Host-side combination of the per-core [1,32] partial vectors."""
    P = np.stack([np.asarray(p).reshape(-1).astype(np.float64)
                  for p in partials])  # [8,32]
    focal = P[:, K_FOCAL].sum() / (B * HW)
    contrast = 0.5 * P[:, K_CONTRAST].sum() / BF

    circ_total = 0.0
    for c in range(NCORES):
        for b in range(BP):
            area = P[c, K_AREA + b]
            ex = P[c, K_EX + b] + P[c, K_EXB + b]
            ey = P[c, K_EY + 2 * b] + P[c, K_EY + 2 * b + 1]
            per = ex + ey
            if area > 0 and per > 0:
                circv = 4.0 * np.pi * area / max(per, 1e-12) ** 2
                circ_total += (circv - 1.0) ** 2
    circ = 0.1 * circ_total / B

    S = P[:, K_S:K_S + 3].sum(axis=0)
    I = P[:, K_I:K_I + 3].sum(axis=0)
    cons_total = 0.0
    for k, (i, j) in enumerate(((0, 1), (0, 2), (1, 2))):
        union = S[i] + S[j] - I[k]
        iou = I[k] / (union + 1e-6)
        cons_total += max(0.6 - iou, 0.0)
    consensus = 0.3 * cons_total / 3.0

    return np.float32(focal + contrast + circ + consensus)


_CACHED_NC = None


def _get_nc():
    global _CACHED_NC
    if _CACHED_NC is None:
        _CACHED_NC = _build_nc()
    return _CACHED_NC


def kernel(logits, target, features, masks, method_preds):
    logits = np.asarray(logits, dtype=np.float32)
    target = np.asarray(target, dtype=np.int32)
    features = np.asarray(features, dtype=np.float32)
    masks = np.asarray(masks, dtype=np.float32)
    method_preds = np.asarray(method_preds, dtype=np.float32)

    in_maps = _host_inputs(logits, target, features, masks, method_preds)
    res = run_bass_kernel_spmd(_get_nc(), in_maps, list(range(NCORES)))
    partials = [res.results[c]["partials"] for c in range(NCORES)]
    return _combine(partials)

